# revision 3
# baseline (speedup 1.0000x reference)
"""Trainium2 Bass kernel for nn_BipartiteGraph1d (gnn_message_passing).

Reference computation (N=16384 rows, D=1024 features, L=num_layers=8):
    history[0] = x
    for i in 1..L-1:
        y = mean_j( history[j] @ m(i,j) )   j in 0..i-1, k = i-j-1
            m(i,j) = att_source[k]                    (i even, j even)
                     target_to_source * att_source[k] (i even, j odd)
                     source_to_target * att_target[k] (i odd,  j even)
                     att_target[k]                    (i odd,  j odd)
        history.append(layernorm(relu(y)))
    out = stack(history[-2:])                         (2, N, D)

Strategy (8 NeuronCores, data-parallel over rows):
  * each core gets 2048 rows, processed in 512-row blocks whose full layer
    history lives in SBUF as PE-transposed tiles hT[j] = h_j.T ([D, 512]).
  * per layer, the mean over j is accumulated directly in PSUM across all
    (j, k-chunk) contributions: 8 psum banks = 4 row-chunks x 2 dout-chunks.
  * weights stream from HBM as [128, 1024] chunks (moving operand of the
    matmul), float32r dtype -> 1 cycle/row on the PE (fp32 data, TF32-class
    rounding, ~1e-4 matmul error).
  * derived matrices (elementwise products with source_to_target /
    target_to_source) are precomputed once per core into DRAM scratch.
  * relu+layernorm run natively per-row (rows on partitions) on ACT/DVE;
    normalized output is PE-transposed back into the SBUF history.
"""

import numpy as np

_CACHE = {}


def _build(L, rows_per_core, D, S, block, num_devices):
    import concourse.tile as tile
    import concourse.mybir as mybir
    from concourse import bacc
    from contextlib import ExitStack

    F32R = mybir.dt.float32r
    F32 = mybir.dt.float32
    Relu = mybir.ActivationFunctionType.Relu
    Sqrt = mybir.ActivationFunctionType.Sqrt

    assert D == 1024, "layout hardcodes D=1024"
    assert rows_per_core % block == 0 and block % 128 == 0
    assert 2 <= L <= S + 1
    KC = D // 128          # contraction chunks per matrix
    RC = block // 128      # row chunks per block
    CC = D // 512          # dout chunks (psum bank width)
    NBLK = rows_per_core // block

    nc = bacc.Bacc("TRN2", target_bir_lowering=False, debug=False,
                   num_devices=num_devices)
    x_d = nc.dram_tensor("x", [rows_per_core, D], F32R, kind="ExternalInput").ap()
    s2t_d = nc.dram_tensor("source_to_target", [D, D], F32R, kind="ExternalInput").ap()
    t2s_d = nc.dram_tensor("target_to_source", [D, D], F32R, kind="ExternalInput").ap()
    As_d = nc.dram_tensor("att_source", [S, D, D], F32R, kind="ExternalInput").ap()
    At_d = nc.dram_tensor("att_target", [S, D, D], F32R, kind="ExternalInput").ap()
    id_d = nc.dram_tensor("ident", [128, 128], F32R, kind="ExternalInput").ap()
    out_d = nc.dram_tensor("out", [2, rows_per_core, D], F32R,
                           kind="ExternalOutput").ap()

    # derived matrices needed: k = i-j-1 (always < S here, so k % S == k)
    need_b = sorted({i - j - 1 for i in range(1, L) for j in range(i)
                     if i % 2 == 0 and j % 2 == 1})
    need_c = sorted({i - j - 1 for i in range(1, L) for j in range(i)
                     if i % 2 == 1 and j % 2 == 0})
    bidx = {k: n for n, k in enumerate(need_b)}
    cidx = {k: n for n, k in enumerate(need_c)}

    with tile.TileContext(nc) as tc, ExitStack() as ctx:
        cst = ctx.enter_context(tc.tile_pool(name="cst", bufs=1))
        hist = ctx.enter_context(tc.tile_pool(name="hist", bufs=1))
        wp = ctx.enter_context(tc.tile_pool(name="wp", bufs=6))
        zp = ctx.enter_context(tc.tile_pool(name="zp", bufs=3))
        hp = ctx.enter_context(tc.tile_pool(name="hp", bufs=4))
        sp = ctx.enter_context(tc.tile_pool(name="sp", bufs=6))
        ps = ctx.enter_context(tc.tile_pool(name="ps", bufs=8, space="PSUM"))
        dramp = ctx.enter_context(tc.tile_pool(name="dramp", bufs=1, space="DRAM"))

        ident = cst.tile([128, 128], F32R)
        nc.sync.dma_start(ident, id_d)
        eps_t = cst.tile([128, 1], F32)
        nc.vector.memset(eps_t, 1e-5)

        bmat = None
        cmat = None
        if need_b:
            bmat = dramp.tile([len(need_b), D, D], F32R, tag="bmat")
        if need_c:
            cmat = dramp.tile([len(need_c), D, D], F32R, tag="cmat")

        # ---- precompute derived weight matrices into DRAM scratch ----
        for ks, att, mult, dstm in ((need_b, As_d, t2s_d, bmat),
                                    (need_c, At_d, s2t_d, cmat)):
            for n, k in enumerate(ks):
                for kc in range(KC):
                    a_t = wp.tile([128, D], F32R, tag="w")
                    nc.sync.dma_start(a_t, att[k, kc * 128:(kc + 1) * 128, :])
                    m_t = wp.tile([128, D], F32R, tag="w")
                    nc.sync.dma_start(m_t, mult[kc * 128:(kc + 1) * 128, :])
                    d_t = wp.tile([128, D], F32R, tag="w")
                    nc.vector.tensor_mul(d_t, a_t, m_t)
                    nc.sync.dma_start(dstm[n, kc * 128:(kc + 1) * 128, :], d_t)

        def wsrc(i, j):
            k = i - j - 1
            if i % 2 == 0 and j % 2 == 0:
                return As_d[k]
            if i % 2 == 0:
                return bmat[bidx[k]]
            if j % 2 == 0:
                return cmat[cidx[k]]
            return At_d[k]

        for b in range(NBLK):
            hT = [hist.tile([128, KC, block], F32R, tag=f"hT{j}", name=f"hT{j}")
                  for j in range(L - 1)]

            def transpose_into(dst_hT, src_tile, r):
                # src [128 rows, D] -> dst[:, dc, r*128:(r+1)*128] for all dc
                for half in range(KC // 4):
                    tp = ps.tile([128, 4, 128], F32R, tag="acc")
                    for q in range(4):
                        dc = half * 4 + q
                        nc.tensor.transpose(
                            tp[:, q, :], src_tile[:, dc * 128:(dc + 1) * 128],
                            ident)
                    nc.scalar.copy(
                        dst_hT[:, half * 4:half * 4 + 4, r * 128:(r + 1) * 128],
                        tp)

            # history[0] = x (transposed into SBUF)
            for r in range(RC):
                xt = hp.tile([128, D], F32R, tag="h")
                row0 = b * block + r * 128
                nc.sync.dma_start(xt, x_d[row0:row0 + 128, :])
                transpose_into(hT[0], xt, r)

            for i in range(1, L):
                jks = [(j, kc) for j in range(i) for kc in range(KC)]
                y = [[ps.tile([128, 512], F32, tag="acc", name=f"y{r}_{c}")
                      for c in range(CC)] for r in range(RC)]
                for n, (j, kc) in enumerate(jks):
                    w_t = wp.tile([128, D], F32R, tag="w")
                    nc.sync.dma_start(w_t, wsrc(i, j)[kc * 128:(kc + 1) * 128, :])
                    for r in range(RC):
                        lhsT = hT[j][:, kc, r * 128:(r + 1) * 128]
                        for c in range(CC):
                            nc.tensor.matmul(
                                y[r][c], lhsT=lhsT,
                                rhs=w_t[:, c * 512:(c + 1) * 512],
                                start=(n == 0), stop=(n == len(jks) - 1))
                inv = 1.0 / i
                for r in range(RC):
                    z = zp.tile([128, D], F32, tag="z")
                    for c in range(CC):
                        nc.scalar.activation(z[:, c * 512:(c + 1) * 512],
                                             y[r][c], Relu, scale=inv)
                    st = sp.tile([128, CC, 6], F32, tag="st")
                    for c in range(CC):
                        nc.vector.bn_stats(st[:, c, :], z[:, c * 512:(c + 1) * 512])
                    mv = sp.tile([128, 2], F32, tag="mv")
                    nc.vector.bn_aggr(mv, st)
                    rstd = sp.tile([128, 1], F32, tag="rs")
                    nc.scalar.activation(rstd, mv[:, 1:2], Sqrt, bias=eps_t)
                    nc.vector.reciprocal(rstd, rstd)
                    h = hp.tile([128, D], F32R, tag="h")
                    nc.vector.tensor_scalar(
                        out=h, in0=z, scalar1=mv[:, 0:1], scalar2=rstd,
                        op0=mybir.AluOpType.subtract, op1=mybir.AluOpType.mult)
                    oi = i - (L - 2)
                    if oi >= 0:
                        row0 = b * block + r * 128
                        nc.sync.dma_start(out_d[oi, row0:row0 + 128, :], h)
                    if i < L - 1:
                        transpose_into(hT[i], h, r)

    nc.compile()
    return nc


def kernel(x, source_to_target, target_to_source, att_source, att_target,
           num_layers):
    from concourse.bass_utils import run_bass_kernel_spmd

    x = np.ascontiguousarray(np.asarray(x, dtype=np.float32))
    s2t = np.ascontiguousarray(np.asarray(source_to_target, dtype=np.float32))
    t2s = np.ascontiguousarray(np.asarray(target_to_source, dtype=np.float32))
    As = np.ascontiguousarray(np.asarray(att_source, dtype=np.float32))
    At = np.ascontiguousarray(np.asarray(att_target, dtype=np.float32))
    L = int(num_layers)

    N, D = x.shape
    S = As.shape[0]
    n_cores = 8
    assert N % n_cores == 0
    rows = N // n_cores
    block = 512 if rows % 512 == 0 else 128

    key = (L, rows, D, S, block, n_cores)
    if key not in _CACHE:
        _CACHE[key] = _build(L, rows, D, S, block, n_cores)
    nc = _CACHE[key]

    ident = np.eye(128, dtype=np.float32)
    in_maps = [
        {
            "x": x[c * rows:(c + 1) * rows],
            "source_to_target": s2t,
            "target_to_source": t2s,
            "att_source": As,
            "att_target": At,
            "ident": ident,
        }
        for c in range(n_cores)
    ]
    res = run_bass_kernel_spmd(nc, in_maps, list(range(n_cores))).results
    out = np.concatenate([res[c]["out"] for c in range(n_cores)], axis=1)
    if L == 2:
        out[0] = x  # history[-2] is the input itself
    return out.astype(np.float32, copy=False)


# revision 5
# speedup vs baseline: 1.0010x; 1.0010x over previous
"""Trainium2 Bass kernel for nn_BipartiteGraph1d (gnn_message_passing).

Reference computation (N=16384 rows, D=1024 features, L=num_layers=8):
    history[0] = x
    for i in 1..L-1:
        y = mean_j( history[j] @ m(i,j) )   j in 0..i-1, k = i-j-1
            m(i,j) = att_source[k]                    (i even, j even)
                     target_to_source * att_source[k] (i even, j odd)
                     source_to_target * att_target[k] (i odd,  j even)
                     att_target[k]                    (i odd,  j odd)
        history.append(layernorm(relu(y)))
    out = stack(history[-2:])                         (2, N, D)

Strategy (8 NeuronCores, data-parallel over rows):
  * each core gets 2048 rows, processed in 512-row blocks whose full layer
    history lives in SBUF as PE-transposed tiles hT[j] = h_j.T ([D, 512]).
  * per layer, the mean over j is accumulated directly in PSUM across all
    (j, k-chunk) contributions: 8 psum banks = 4 row-chunks x 2 dout-chunks.
  * weights stream from HBM as [128, 1024] chunks (moving operand of the
    matmul), float32r dtype -> 1 cycle/row on the PE (fp32 data, TF32-class
    rounding, ~1e-4 matmul error).
  * derived matrices (elementwise products with source_to_target /
    target_to_source) are precomputed once per core into DRAM scratch.
  * relu+layernorm run natively per-row (rows on partitions) on ACT/DVE;
    normalized output is PE-transposed back into the SBUF history.
"""

import numpy as np

_CACHE = {}


def _build(L, rows_per_core, D, S, block, num_devices):
    import concourse.tile as tile
    import concourse.mybir as mybir
    from concourse import bacc
    from contextlib import ExitStack

    F32R = mybir.dt.float32r
    F32 = mybir.dt.float32
    Relu = mybir.ActivationFunctionType.Relu
    Sqrt = mybir.ActivationFunctionType.Sqrt

    assert D == 1024, "layout hardcodes D=1024"
    assert rows_per_core % block == 0 and block % 128 == 0
    assert 2 <= L <= S + 1
    KC = D // 128          # contraction chunks per matrix
    RC = block // 128      # row chunks per block
    CC = D // 512          # dout chunks (psum bank width)
    NBLK = rows_per_core // block

    nc = bacc.Bacc("TRN2", target_bir_lowering=False, debug=False,
                   num_devices=num_devices)
    x_d = nc.dram_tensor("x", [rows_per_core, D], F32R, kind="ExternalInput").ap()
    s2t_d = nc.dram_tensor("source_to_target", [D, D], F32R, kind="ExternalInput").ap()
    t2s_d = nc.dram_tensor("target_to_source", [D, D], F32R, kind="ExternalInput").ap()
    As_d = nc.dram_tensor("att_source", [S, D, D], F32R, kind="ExternalInput").ap()
    At_d = nc.dram_tensor("att_target", [S, D, D], F32R, kind="ExternalInput").ap()
    id_d = nc.dram_tensor("ident", [128, 128], F32R, kind="ExternalInput").ap()
    out_d = nc.dram_tensor("out", [2, rows_per_core, D], F32R,
                           kind="ExternalOutput").ap()

    # derived matrices needed: k = i-j-1 (always < S here, so k % S == k)
    need_b = sorted({i - j - 1 for i in range(1, L) for j in range(i)
                     if i % 2 == 0 and j % 2 == 1})
    need_c = sorted({i - j - 1 for i in range(1, L) for j in range(i)
                     if i % 2 == 1 and j % 2 == 0})
    bidx = {k: n for n, k in enumerate(need_b)}
    cidx = {k: n for n, k in enumerate(need_c)}

    with tile.TileContext(nc) as tc, ExitStack() as ctx:
        cst = ctx.enter_context(tc.tile_pool(name="cst", bufs=1))
        hist = ctx.enter_context(tc.tile_pool(name="hist", bufs=1))
        wp = ctx.enter_context(tc.tile_pool(name="wp", bufs=6))
        zp = ctx.enter_context(tc.tile_pool(name="zp", bufs=3))
        hp = ctx.enter_context(tc.tile_pool(name="hp", bufs=4))
        sp = ctx.enter_context(tc.tile_pool(name="sp", bufs=6))
        pp = ctx.enter_context(tc.tile_pool(name="pp", bufs=6))
        ps = ctx.enter_context(tc.tile_pool(name="ps", bufs=8, space="PSUM"))
        dramp = ctx.enter_context(tc.tile_pool(name="dramp", bufs=1, space="DRAM"))

        ident = cst.tile([128, 128], F32R)
        nc.sync.dma_start(ident, id_d)
        eps_t = cst.tile([128, 1], F32)
        nc.vector.memset(eps_t, 1e-5)
        dummy = cst.tile([128, 128], mybir.dt.bfloat16)
        nc.vector.memset(dummy, 0.0)

        # scratch: one DRAM tile per derived matrix so dependency tracking
        # is per-matrix (layer 1 only waits for C_0, not the whole batch)
        bmat_t = {k: dramp.tile([D, D], F32R, tag=f"bm{k}", name=f"bm{k}")
                  for k in need_b}
        cmat_t = {k: dramp.tile([D, D], F32R, tag=f"cm{k}", name=f"cm{k}")
                  for k in need_c}

        # emit the block-0 input loads first so they are at the head of the
        # scalar-engine DMA ring, ahead of the precompute streams
        x0_tiles = []
        for r in range(RC):
            xt0 = hp.tile([128, D], F32R, tag="h", name=f"x0_{r}")
            nc.scalar.dma_start(xt0, x_d[r * 128:(r + 1) * 128, :])
            x0_tiles.append(xt0)

        # ---- precompute derived weight matrices into DRAM scratch ----
        # in first-use order: C_k first used at layer k+1, B_k at layer k+2.
        # DMAs ride the scalar (ACT) HWDGE ring, the products run on the
        # otherwise-idle GpSimd engine: nothing here contends with the
        # weight stream (sync ring) or the LN pipeline (DVE/ACT compute).
        pre = sorted([("c", k) for k in need_c] + [("b", k) for k in need_b],
                     key=lambda t: t[1] + (1 if t[0] == "c" else 2))
        for kind, k in pre:
            att, mult, dstm = ((As_d, t2s_d, bmat_t[k]) if kind == "b"
                               else (At_d, s2t_d, cmat_t[k]))
            for kc in range(KC):
                a_t = pp.tile([128, D], F32R, tag="pre", name="pa")
                nc.scalar.dma_start(a_t, att[k, kc * 128:(kc + 1) * 128, :])
                m_t = pp.tile([128, D], F32R, tag="pre", name="pm")
                nc.scalar.dma_start(m_t, mult[kc * 128:(kc + 1) * 128, :])
                d_t = pp.tile([128, D], F32R, tag="pre", name="pd")
                nc.gpsimd.tensor_mul(d_t, a_t, m_t)
                nc.scalar.dma_start(dstm[kc * 128:(kc + 1) * 128, :], d_t)

        def wsrc(i, j):
            k = i - j - 1
            if i % 2 == 0 and j % 2 == 0:
                return As_d[k]
            if i % 2 == 0:
                return bmat_t[k]
            if j % 2 == 0:
                return cmat_t[k]
            return At_d[k]

        def burst(n):
            # keep the PE's activity monitor busy across short dependency
            # stalls: idle >3.4us halves the PE clock for >10us (HAM gate)
            for _ in range(n):
                nc.tensor.ldweights(dummy)

        for b in range(NBLK):
            hT = [hist.tile([128, KC, block], F32R, tag=f"hT{j}", name=f"hT{j}")
                  for j in range(L - 1)]

            def transpose_into(dst_hT, src_tile, r):
                # src [128 rows, D] -> dst[:, dc, r*128:(r+1)*128] for all dc
                for half in range(KC // 4):
                    tp = ps.tile([128, 4, 128], F32R, tag="acc")
                    for q in range(4):
                        dc = half * 4 + q
                        nc.tensor.transpose(
                            tp[:, q, :], src_tile[:, dc * 128:(dc + 1) * 128],
                            ident)
                    nc.scalar.copy(
                        dst_hT[:, half * 4:half * 4 + 4, r * 128:(r + 1) * 128],
                        tp)

            # history[0] = x (transposed into SBUF)
            for r in range(RC):
                if b == 0:
                    xt = x0_tiles[r]
                else:
                    xt = hp.tile([128, D], F32R, tag="h", name="xt")
                    row0 = b * block + r * 128
                    nc.scalar.dma_start(xt, x_d[row0:row0 + 128, :])
                transpose_into(hT[0], xt, r)
                burst(8)

            for i in range(1, L):
                jks = [(j, kc) for j in range(i) for kc in range(KC)]
                y = [[ps.tile([128, 512], F32, tag="acc", name=f"y{r}_{c}")
                      for c in range(CC)] for r in range(RC)]
                for n, (j, kc) in enumerate(jks):
                    w_t = wp.tile([128, D], F32R, tag="w")
                    nc.sync.dma_start(w_t, wsrc(i, j)[kc * 128:(kc + 1) * 128, :])
                    for r in range(RC):
                        lhsT = hT[j][:, kc, r * 128:(r + 1) * 128]
                        for c in range(CC):
                            nc.tensor.matmul(
                                y[r][c], lhsT=lhsT,
                                rhs=w_t[:, c * 512:(c + 1) * 512],
                                start=(n == 0), stop=(n == len(jks) - 1))
                burst(24)
                inv = 1.0 / i
                for r in range(RC):
                    z = zp.tile([128, D], F32, tag="z")
                    for c in range(CC):
                        nc.scalar.activation(z[:, c * 512:(c + 1) * 512],
                                             y[r][c], Relu, scale=inv)
                    st = sp.tile([128, CC, 6], F32, tag="st")
                    for c in range(CC):
                        nc.vector.bn_stats(st[:, c, :], z[:, c * 512:(c + 1) * 512])
                    mv = sp.tile([128, 2], F32, tag="mv")
                    nc.vector.bn_aggr(mv, st)
                    rstd = sp.tile([128, 1], F32, tag="rs")
                    nc.scalar.activation(rstd, mv[:, 1:2], Sqrt, bias=eps_t)
                    nc.vector.reciprocal(rstd, rstd)
                    h = hp.tile([128, D], F32R, tag="h")
                    nc.vector.tensor_scalar(
                        out=h, in0=z, scalar1=mv[:, 0:1], scalar2=rstd,
                        op0=mybir.AluOpType.subtract, op1=mybir.AluOpType.mult)
                    oi = i - (L - 2)
                    if oi >= 0:
                        row0 = b * block + r * 128
                        nc.scalar.dma_start(out_d[oi, row0:row0 + 128, :], h)
                    if i < L - 1:
                        transpose_into(hT[i], h, r)
                        burst(8)

    nc.compile()
    return nc


def kernel(x, source_to_target, target_to_source, att_source, att_target,
           num_layers):
    from concourse.bass_utils import run_bass_kernel_spmd

    x = np.ascontiguousarray(np.asarray(x, dtype=np.float32))
    s2t = np.ascontiguousarray(np.asarray(source_to_target, dtype=np.float32))
    t2s = np.ascontiguousarray(np.asarray(target_to_source, dtype=np.float32))
    As = np.ascontiguousarray(np.asarray(att_source, dtype=np.float32))
    At = np.ascontiguousarray(np.asarray(att_target, dtype=np.float32))
    L = int(num_layers)

    N, D = x.shape
    S = As.shape[0]
    n_cores = 8
    assert N % n_cores == 0
    rows = N // n_cores
    block = 512 if rows % 512 == 0 else 128

    key = (L, rows, D, S, block, n_cores)
    if key not in _CACHE:
        _CACHE[key] = _build(L, rows, D, S, block, n_cores)
    nc = _CACHE[key]

    ident = np.eye(128, dtype=np.float32)
    in_maps = [
        {
            "x": x[c * rows:(c + 1) * rows],
            "source_to_target": s2t,
            "target_to_source": t2s,
            "att_source": As,
            "att_target": At,
            "ident": ident,
        }
        for c in range(n_cores)
    ]
    res = run_bass_kernel_spmd(nc, in_maps, list(range(n_cores))).results
    out = np.concatenate([res[c]["out"] for c in range(n_cores)], axis=1)
    if L == 2:
        out[0] = x  # history[-2] is the input itself
    return out.astype(np.float32, copy=False)


# revision 6
# speedup vs baseline: 1.0306x; 1.0296x over previous
"""Trainium2 Bass kernel for nn_BipartiteGraph1d (gnn_message_passing).

Reference computation (N=16384 rows, D=1024 features, L=num_layers=8):
    history[0] = x
    for i in 1..L-1:
        y = mean_j( history[j] @ m(i,j) )   j in 0..i-1, k = i-j-1
            m(i,j) = att_source[k]                    (i even, j even)
                     target_to_source * att_source[k] (i even, j odd)
                     source_to_target * att_target[k] (i odd,  j even)
                     att_target[k]                    (i odd,  j odd)
        history.append(layernorm(relu(y)))
    out = stack(history[-2:])                         (2, N, D)

Strategy (8 NeuronCores, data-parallel over rows):
  * each core gets 2048 rows, processed in 512-row blocks whose full layer
    history lives in SBUF as PE-transposed tiles hT[j] = h_j.T ([D, 512]).
  * per layer, the mean over j is accumulated directly in PSUM across all
    (j, k-chunk) contributions: 8 psum banks = 4 row-chunks x 2 dout-chunks.
  * weights stream from HBM as [128, 1024] chunks (moving operand of the
    matmul), float32r dtype -> 1 cycle/row on the PE (fp32 data, TF32-class
    rounding, ~1e-4 matmul error), on the sync-engine HWDGE ring.
  * derived matrices (elementwise products with source_to_target /
    target_to_source) are precomputed into per-matrix DRAM scratch tiles on
    the DVE, with emission interleaved so each matrix is produced just
    before the layer that first consumes it; x loads / outputs / precompute
    streams ride the scalar-engine HWDGE ring so they never queue behind
    the weight stream.
  * per layer the j's are ordered so the freshest derived matrix is
    second-to-last and the freshest history tile last - maximum slack for
    the LN->transpose pipeline and the precompute.
  * relu+layernorm run natively per-row (rows on partitions) on ACT/DVE;
    normalized output is PE-transposed back into the SBUF history.
  * zero-weight "warm" matmuls at layer boundaries keep the PE activity
    monitor from halving the clock (idle >3.4us => 1.2 GHz for >10us).
"""

import numpy as np

_CACHE = {}

WARM_TAIL = 16   # zero-matmuls right after each layer's real stream
WARM_TP = 4      # zero-matmuls after each transpose group


def _build(L, rows_per_core, D, S, block, num_devices):
    import concourse.tile as tile
    import concourse.mybir as mybir
    from concourse import bacc
    from contextlib import ExitStack

    F32R = mybir.dt.float32r
    F32 = mybir.dt.float32
    Relu = mybir.ActivationFunctionType.Relu
    Sqrt = mybir.ActivationFunctionType.Sqrt

    assert D == 1024, "layout hardcodes D=1024"
    assert rows_per_core % block == 0 and block % 128 == 0
    assert 2 <= L <= S + 1
    KC = D // 128          # contraction chunks per matrix
    RC = block // 128      # row chunks per block
    CC = D // 512          # dout chunks (psum bank width)
    NBLK = rows_per_core // block

    nc = bacc.Bacc("TRN2", target_bir_lowering=False, debug=False,
                   num_devices=num_devices)
    x_d = nc.dram_tensor("x", [rows_per_core, D], F32R, kind="ExternalInput").ap()
    s2t_d = nc.dram_tensor("source_to_target", [D, D], F32R, kind="ExternalInput").ap()
    t2s_d = nc.dram_tensor("target_to_source", [D, D], F32R, kind="ExternalInput").ap()
    As_d = nc.dram_tensor("att_source", [S, D, D], F32R, kind="ExternalInput").ap()
    At_d = nc.dram_tensor("att_target", [S, D, D], F32R, kind="ExternalInput").ap()
    id_d = nc.dram_tensor("ident", [128, 128], F32R, kind="ExternalInput").ap()
    zero_d = nc.dram_tensor("zeros", [128, 512], F32R, kind="ExternalInput").ap()
    out_d = nc.dram_tensor("out", [2, rows_per_core, D], F32R,
                           kind="ExternalOutput").ap()

    # derived matrices needed: k = i-j-1 (always < S here, so k % S == k)
    need_b = sorted({i - j - 1 for i in range(1, L) for j in range(i)
                     if i % 2 == 0 and j % 2 == 1})
    need_c = sorted({i - j - 1 for i in range(1, L) for j in range(i)
                     if i % 2 == 1 and j % 2 == 0})

    with tile.TileContext(nc) as tc, ExitStack() as ctx:
        cst = ctx.enter_context(tc.tile_pool(name="cst", bufs=1))
        hist = ctx.enter_context(tc.tile_pool(name="hist", bufs=1))
        wp = ctx.enter_context(tc.tile_pool(name="wp", bufs=6))
        zp = ctx.enter_context(tc.tile_pool(name="zp", bufs=3))
        hp = ctx.enter_context(tc.tile_pool(name="hp", bufs=6))
        sp = ctx.enter_context(tc.tile_pool(name="sp", bufs=6))
        pp = ctx.enter_context(tc.tile_pool(name="pp", bufs=6))
        ps = ctx.enter_context(tc.tile_pool(name="ps", bufs=8, space="PSUM"))
        dramp = ctx.enter_context(tc.tile_pool(name="dramp", bufs=1, space="DRAM"))

        ident = cst.tile([128, 128], F32R)
        nc.scalar.dma_start(ident, id_d)
        zeros = cst.tile([128, 512], F32R)
        nc.scalar.dma_start(zeros, zero_d)
        eps_t = cst.tile([128, 1], F32)
        nc.vector.memset(eps_t, 1e-5)

        # one DRAM scratch tile per derived matrix: dependency tracking is
        # then per-matrix, so a layer only waits for the matrix it reads.
        bmat_t = {k: dramp.tile([D, D], F32R, tag=f"bm{k}", name=f"bm{k}")
                  for k in need_b}
        cmat_t = {k: dramp.tile([D, D], F32R, tag=f"cm{k}", name=f"cm{k}")
                  for k in need_c}

        # block-0 x loads first: head of the scalar DMA ring
        x0_tiles = []
        for r in range(RC):
            xt0 = hp.tile([128, D], F32R, tag="h", name=f"x0_{r}")
            nc.scalar.dma_start(xt0, x_d[r * 128:(r + 1) * 128, :])
            x0_tiles.append(xt0)

        def precompute(kind, k):
            att, mult, dstm = ((As_d, t2s_d, bmat_t[k]) if kind == "b"
                               else (At_d, s2t_d, cmat_t[k]))
            for kc in range(KC):
                a_t = pp.tile([128, D], F32R, tag="pre", name="pa")
                nc.scalar.dma_start(a_t, att[k, kc * 128:(kc + 1) * 128, :])
                m_t = pp.tile([128, D], F32R, tag="pre", name="pm")
                nc.scalar.dma_start(m_t, mult[kc * 128:(kc + 1) * 128, :])
                d_t = pp.tile([128, D], F32R, tag="pre", name="pd")
                nc.vector.tensor_mul(d_t, a_t, m_t)
                nc.scalar.dma_start(dstm[kc * 128:(kc + 1) * 128, :], d_t)

        # first-use layer 1 and 2 matrices up front
        if 0 in need_c:
            precompute("c", 0)
        if 0 in need_b:
            precompute("b", 0)

        def wsrc(i, j):
            k = i - j - 1
            if i % 2 == 0 and j % 2 == 0:
                return As_d[k]
            if i % 2 == 0:
                return bmat_t[k]
            if j % 2 == 0:
                return cmat_t[k]
            return At_d[k]

        for b in range(NBLK):
            hT = [hist.tile([128, KC, block], F32R, tag=f"hT{j}", name=f"hT{j}")
                  for j in range(L - 1)]

            def warm(n, y_tile, hT0=None):
                # zero-contribution matmuls: keep the PE array active (HAM
                # clock gate) across the LN/transpose dependency stall.
                src = hT0 if hT0 is not None else hT[0]
                for _ in range(n):
                    nc.tensor.matmul(y_tile, lhsT=src[:, 0, 0:128],
                                     rhs=zeros, start=False, stop=True,
                                     skip_group_check=True)

            def transpose_into(dst_hT, src_tile, r):
                # src [128 rows, D] -> dst[:, dc, r*128:(r+1)*128] for all dc
                for half in range(KC // 4):
                    tp = ps.tile([128, 4, 128], F32R, tag="acc", name="tp")
                    for q in range(4):
                        dc = half * 4 + q
                        nc.tensor.transpose(
                            tp[:, q, :], src_tile[:, dc * 128:(dc + 1) * 128],
                            ident)
                    nc.scalar.copy(
                        dst_hT[:, half * 4:half * 4 + 4, r * 128:(r + 1) * 128],
                        tp)

            # history[0] = x (transposed into SBUF)
            for r in range(RC):
                if b == 0:
                    xt = x0_tiles[r]
                else:
                    xt = hp.tile([128, D], F32R, tag="h", name="xt")
                    row0 = b * block + r * 128
                    nc.scalar.dma_start(xt, x_d[row0:row0 + 128, :])
                transpose_into(hT[0], xt, r)

            for i in range(1, L):
                # j order: middle js, then freshest derived matrix user,
                # then the freshest-history j last.
                js = list(range(1, i - 1)) + [0, i - 1] if i >= 2 else [0]
                jks = [(j, kc) for j in js for kc in range(KC)]
                y = [[ps.tile([128, 512], F32, tag="acc", name=f"y{r}_{c}")
                      for c in range(CC)] for r in range(RC)]
                for n, (j, kc) in enumerate(jks):
                    w_t = wp.tile([128, D], F32R, tag="w", name="wt")
                    nc.sync.dma_start(w_t, wsrc(i, j)[kc * 128:(kc + 1) * 128, :])
                    for r in range(RC):
                        lhsT = hT[j][:, kc, r * 128:(r + 1) * 128]
                        for c in range(CC):
                            nc.tensor.matmul(
                                y[r][c], lhsT=lhsT,
                                rhs=w_t[:, c * 512:(c + 1) * 512],
                                start=(n == 0), stop=(n == len(jks) - 1))
                warm(WARM_TAIL, y[RC - 1][CC - 1])
                inv = 1.0 / i
                for r in range(RC):
                    z = zp.tile([128, D], F32, tag="z", name="z")
                    for c in range(CC):
                        nc.scalar.activation(z[:, c * 512:(c + 1) * 512],
                                             y[r][c], Relu, scale=inv)
                    st = sp.tile([128, CC, 6], F32, tag="st", name="st")
                    for c in range(CC):
                        nc.vector.bn_stats(st[:, c, :], z[:, c * 512:(c + 1) * 512])
                    mv = sp.tile([128, 2], F32, tag="mv", name="mv")
                    nc.vector.bn_aggr(mv, st)
                    rstd = sp.tile([128, 1], F32, tag="rs", name="rs")
                    nc.scalar.activation(rstd, mv[:, 1:2], Sqrt, bias=eps_t)
                    nc.vector.reciprocal(rstd, rstd)
                    h = hp.tile([128, D], F32R, tag="h", name="h")
                    nc.vector.tensor_scalar(
                        out=h, in0=z, scalar1=mv[:, 0:1], scalar2=rstd,
                        op0=mybir.AluOpType.subtract, op1=mybir.AluOpType.mult)
                    oi = i - (L - 2)
                    if oi >= 0:
                        row0 = b * block + r * 128
                        nc.scalar.dma_start(out_d[oi, row0:row0 + 128, :], h)
                    if i < L - 1:
                        transpose_into(hT[i], h, r)
                        if r < RC - 1:
                            warm(WARM_TP, y[RC - 1][CC - 1])

                # block 0: emit the precompute for the matrix first used at
                # layer i+1 (overlaps with layer i+1's early-j matmuls)
                if b == 0 and i + 1 < L:
                    if i % 2 == 0 and i in need_c:
                        precompute("c", i)
                    elif i % 2 == 1 and i >= 3 and (i - 1) in need_b:
                        precompute("b", i - 1)

    nc.compile()
    return nc


def kernel(x, source_to_target, target_to_source, att_source, att_target,
           num_layers):
    from concourse.bass_utils import run_bass_kernel_spmd

    x = np.ascontiguousarray(np.asarray(x, dtype=np.float32))
    s2t = np.ascontiguousarray(np.asarray(source_to_target, dtype=np.float32))
    t2s = np.ascontiguousarray(np.asarray(target_to_source, dtype=np.float32))
    As = np.ascontiguousarray(np.asarray(att_source, dtype=np.float32))
    At = np.ascontiguousarray(np.asarray(att_target, dtype=np.float32))
    L = int(num_layers)

    N, D = x.shape
    S = As.shape[0]
    n_cores = 8
    assert N % n_cores == 0
    rows = N // n_cores
    block = 512 if rows % 512 == 0 else 128

    key = (L, rows, D, S, block, n_cores)
    if key not in _CACHE:
        _CACHE[key] = _build(L, rows, D, S, block, n_cores)
    nc = _CACHE[key]

    ident = np.eye(128, dtype=np.float32)
    zeros = np.zeros((128, 512), dtype=np.float32)
    in_maps = [
        {
            "x": x[c * rows:(c + 1) * rows],
            "source_to_target": s2t,
            "target_to_source": t2s,
            "att_source": As,
            "att_target": At,
            "ident": ident,
            "zeros": zeros,
        }
        for c in range(n_cores)
    ]
    res = run_bass_kernel_spmd(nc, in_maps, list(range(n_cores))).results
    out = np.concatenate([res[c]["out"] for c in range(n_cores)], axis=1)
    if L == 2:
        out[0] = x  # history[-2] is the input itself
    return out.astype(np.float32, copy=False)
